# revision 2
# baseline (speedup 1.0000x reference)
# MixGAT layer (GATConv + beta-mix swish) on 8 Trainium2 NeuronCores.
#
# Strategy (dst-node sharding, per spec sharding_hint):
#  - Nodes partitioned across 8 cores by dst id; each core owns N/8 dst rows.
#  - Params (W, att, bias) replicated; each core computes the full projected
#    table xp = x @ W on-device (replicated compute beats collectives here).
#  - Phase A (device): xp (fp16), a_src/a_dst (f32) written to a 512B/row HBM
#    table:  [128 x fp16 xp | 1.0 fp16 | 4 x f32 a_src | 4 x f32 a_dst | pad].
#  - Phase B (device): edges sorted by dst, packed into 32-node groups of
#    9 static 128-edge blocks (6 "lo" + 3 "hi": dma_gather indices are int16,
#    so the table is addressed as two halves).  dma_gather pulls each edge's
#    512B row (edge -> partition).  alpha = lrelu(a_src+a_dst) -> exp on ACT.
#    A sparse per-block weight matrix Mw[e, h*32+c] = exp(alpha[e,h]) (c = dst
#    slot) is built with is_equal/mult on DVE; ONE matmul per block computes
#    both the weighted feature aggregation and the softmax denominators (the
#    baked-in 1.0 column), accumulating over the group's blocks in PSUM.
#    Postproc divides by the denominator and applies the beta-mix swish.
#  - Phase C (device): small gather permutes group-slot rows to node order.
#
# kernel(**inputs) is self-contained: preprocessing is pure numpy, the device
# kernel is built with bass/Tile and run via run_bass_kernel_spmd on cores 0-7.

import math

import numpy as np

import concourse.bass as bass
import concourse.mybir as mybir
import concourse.tile as tile
from concourse import bacc
from concourse.bass_utils import run_bass_kernel_spmd

F32 = mybir.dt.float32
F16 = mybir.dt.float16
I16 = mybir.dt.int16

# problem constants
N_NODES = 50000
IN_DIM = 128
HEADS = 4
OUT_DIM = 32
LEAKY_SLOPE = 0.2
BETA = 0.5
CMIX = 1.2
N_CORES = 8

# static schedule constants
WIN = 32          # dst nodes per group (PSUM slots = HEADS*WIN = 128)
BLK = 128         # edges per block (gather slots -> partitions)
LOB = 6           # lo blocks per group
HIB = 3           # hi blocks per group
GPB = 4           # groups per superblock
SPLIT = 32768     # int16-addressable table split
LO_CAP = LOB * BLK
HI_CAP = HIB * BLK
DEAD = 100.0      # colidx value for dead slots (never equals iota 0..31)
ROW_F16 = 256     # fp16 elements per 512B table row
PC_CHUNK = 2048   # phase C gather chunk (nodes)


class Cfg:
    def __init__(self, n_all, npc, split, nsb, ncc, pd=F16, bias_nonzero=False,
                 n_cores=N_CORES, phases="ABC", blevel=4):
        self.phases = phases
        self.blevel = blevel
        self.n_all = n_all          # total nodes (table rows)
        self.npc = npc              # nodes per core
        self.split = split
        self.nsb = nsb              # superblocks per core
        self.ncc = ncc              # phase C chunks
        self.pd = pd
        self.bias_nonzero = bias_nonzero
        self.n_cores = n_cores


# ---------------------------------------------------------------- host side

def build_nc_adst(n_rows, pd, n_cores):
    """Launch-1 mini kernel: adstv[4, n_rows] = (x_slab @ W @ AD4).T per core."""
    nc = bacc.Bacc("TRN2", target_bir_lowering=False, debug=False,
                   num_devices=n_cores)
    xs_t = nc.dram_tensor("x_slab", [n_rows, IN_DIM], F32, kind="ExternalInput")
    wad_t = nc.dram_tensor("wad_pd", [IN_DIM, HEADS], pd, kind="ExternalInput")
    idf_t = nc.dram_tensor("ident_f32", [128, 128], F32, kind="ExternalInput")
    out_t = nc.dram_tensor("adstv", [HEADS, n_rows], F32, kind="ExternalOutput")
    with tile.TileContext(nc) as tc:
        with (tc.tile_pool(name="c", bufs=1) as cp,
              tc.tile_pool(name="s", bufs=3) as sp,
              tc.tile_pool(name="p", bufs=2, space="PSUM") as pp,
              tc.tile_pool(name="p2", bufs=2, space="PSUM") as pp2):
            wad_c = cp.tile([IN_DIM, HEADS], pd)
            nc.sync.dma_start(wad_c[:], wad_t.ap())
            idf_c = cp.tile([128, 128], F32)
            nc.sync.dma_start(idf_c[:], idf_t.ap())
            n_tiles = (n_rows + 127) // 128
            for t in range(n_tiles):
                n0 = t * 128
                p = min(128, n_rows - n0)
                xt = sp.tile([128, IN_DIM], F32, tag="xt")
                nc.sync.dma_start(xt[:p, :], xs_t.ap()[n0:n0 + p, :])
                xT_ps = pp.tile([128, 128], F32, tag="xT")
                nc.tensor.transpose(out=xT_ps[:, :p], in_=xt[:p, :],
                                    identity=idf_c[:p, :p])
                xT = sp.tile([128, 128], pd, tag="xTs")
                nc.scalar.copy(xT[:, :p], xT_ps[:, :p])
                av_ps = pp2.tile([HEADS, 128], F32, tag="av")
                nc.tensor.matmul(av_ps[:, :p], lhsT=wad_c[:], rhs=xT[:, :p],
                                 start=True, stop=True)
                av = sp.tile([HEADS, 128], F32, tag="avs")
                nc.vector.tensor_copy(av[:, :p], av_ps[:, :p])
                nc.sync.dma_start(out_t.ap()[:, n0:n0 + p], av[:, :p])
    nc.compile()
    return nc


def _wrap16(v):
    """idx vector [S*16] -> dma_gather idx layout [128, S]."""
    s = v.reshape(-1, 16).T                      # [16, S]
    return np.tile(s, (8, 1)).astype(np.int16)   # [128, S]


def preprocess(edge_index, n_all, npc, split, n_cores):
    """Build per-core static schedules. Returns (cfg-ish dict, per-core arrays)."""
    src = np.asarray(edge_index[0], dtype=np.int64)
    dst = np.asarray(edge_index[1], dtype=np.int64)
    loop = np.arange(n_all, dtype=np.int64)
    src = np.concatenate([src, loop])
    dst = np.concatenate([dst, loop])
    order = np.argsort(dst, kind="stable")
    src = src[order]
    dst = dst[order]

    core_bounds = np.searchsorted(dst, np.arange(n_cores + 1) * npc)
    cores = []
    for c in range(n_cores):
        b0, b1 = core_bounds[c], core_bounds[c + 1]
        s = src[b0:b1]
        d = (dst[b0:b1] - c * npc).astype(np.int64)
        lo_mask = s < split
        deg_lo = np.bincount(d[lo_mask], minlength=npc)
        deg_hi = np.bincount(d[~lo_mask], minlength=npc)
        # node -> edge range (d is sorted)
        seg_end = np.cumsum(np.bincount(d, minlength=npc))
        seg_start = seg_end - (deg_lo + deg_hi)

        # greedy 32-node groups under the static caps
        groups = []  # (n0, cnt)
        n = 0
        while n < npc:
            cnt, lo, hi = 0, 0, 0
            while (n + cnt < npc and cnt < WIN
                   and lo + deg_lo[n + cnt] <= LO_CAP
                   and hi + deg_hi[n + cnt] <= HI_CAP):
                lo += deg_lo[n + cnt]
                hi += deg_hi[n + cnt]
                cnt += 1
            assert cnt > 0, "single node exceeds block caps"
            groups.append((n, cnt))
            n += cnt
        g_cnt = len(groups)

        # per-group static block data
        idx_lo = np.zeros((g_cnt, LO_CAP), dtype=np.int64)
        idx_hi = np.zeros((g_cnt, HI_CAP), dtype=np.int64)
        col_lo = np.full((g_cnt, LO_CAP), DEAD, dtype=np.float16)
        col_hi = np.full((g_cnt, HI_CAP), DEAD, dtype=np.float16)
        dn_lo = np.zeros((g_cnt, LO_CAP), dtype=np.int64)
        dn_hi = np.zeros((g_cnt, HI_CAP), dtype=np.int64)
        perm = np.empty(npc, dtype=np.int64)
        for g, (n0, cnt) in enumerate(groups):
            e0, e1 = seg_start[n0], seg_end[n0 + cnt - 1]
            gs = s[e0:e1]
            gd = d[e0:e1] - n0
            m = gs < split
            nl = int(m.sum())
            nh = int((~m).sum())
            idx_lo[g, :nl] = gs[m]
            col_lo[g, :nl] = gd[m].astype(np.float16)
            dn_lo[g, :nl] = c * npc + n0 + gd[m]
            idx_hi[g, :nh] = gs[~m] - split
            col_hi[g, :nh] = gd[~m].astype(np.float16)
            dn_hi[g, :nh] = c * npc + n0 + gd[~m]
            perm[n0:n0 + cnt] = g * WIN + np.arange(cnt)
        cores.append(dict(g_cnt=g_cnt, idx_lo=idx_lo, idx_hi=idx_hi,
                          col_lo=col_lo, col_hi=col_hi, dn_lo=dn_lo,
                          dn_hi=dn_hi, perm=perm))

    g_max = max(c["g_cnt"] for c in cores)
    g_pad = ((g_max + GPB - 1) // GPB) * GPB
    nsb = g_pad // GPB
    ncc = (npc + PC_CHUNK - 1) // PC_CHUNK

    per_core = []
    for c in cores:
        g_cnt = c["g_cnt"]
        il = np.zeros((g_pad, LO_CAP), dtype=np.int64)
        ih = np.zeros((g_pad, HI_CAP), dtype=np.int64)
        cl = np.full((g_pad, LO_CAP), DEAD, dtype=np.float16)
        ch = np.full((g_pad, HI_CAP), DEAD, dtype=np.float16)
        dl = np.zeros((g_pad, LO_CAP), dtype=np.int64)
        dh = np.zeros((g_pad, HI_CAP), dtype=np.int64)
        il[:g_cnt] = c["idx_lo"]
        ih[:g_cnt] = c["idx_hi"]
        cl[:g_cnt] = c["col_lo"]
        ch[:g_cnt] = c["col_hi"]
        dl[:g_cnt] = c["dn_lo"]
        dh[:g_cnt] = c["dn_hi"]

        # gather idx tensors: [NSB, 128, S]
        idx_lo_t = np.stack([_wrap16(il[sb * GPB:(sb + 1) * GPB].reshape(-1))
                             for sb in range(nsb)])
        idx_hi_t = np.stack([_wrap16(ih[sb * GPB:(sb + 1) * GPB].reshape(-1))
                             for sb in range(nsb)])

        # colidx tensor: [NSB, 128, GPB*(LOB+HIB)]  (block cols: 24 lo | 12 hi)
        cl_b = cl.reshape(g_pad, LOB, BLK)
        ch_b = ch.reshape(g_pad, HIB, BLK)
        dl_b = dl.reshape(g_pad, LOB, BLK)
        dh_b = dh.reshape(g_pad, HIB, BLK)
        colidx = np.empty((nsb, 128, GPB * (LOB + HIB)), dtype=np.float16)
        dstn = np.empty((nsb, 128, GPB * (LOB + HIB)), dtype=np.int64)
        for sb in range(nsb):
            sl = slice(sb * GPB, (sb + 1) * GPB)
            colidx[sb, :, :GPB * LOB] = cl_b[sl].reshape(-1, BLK).T
            colidx[sb, :, GPB * LOB:] = ch_b[sl].reshape(-1, BLK).T
            dstn[sb, :, :GPB * LOB] = dl_b[sl].reshape(-1, BLK).T
            dstn[sb, :, GPB * LOB:] = dh_b[sl].reshape(-1, BLK).T

        # phase C perm idx: [NCC, 128, PC_CHUNK//16]
        permidx = np.empty((ncc, 128, PC_CHUNK // 16), dtype=np.int16)
        for k in range(ncc):
            v = np.full(PC_CHUNK, -1, dtype=np.int64)
            n0 = k * PC_CHUNK
            n1 = min(npc, n0 + PC_CHUNK)
            v[:n1 - n0] = c["perm"][n0:n1]
            permidx[k] = _wrap16(v)

        per_core.append(dict(idx_lo=idx_lo_t, idx_hi=idx_hi_t, colidx=colidx,
                             dstn=dstn, permidx=permidx))
    return nsb, ncc, g_pad, per_core


# -------------------------------------------------------------- device side

def build_nc(cfg: Cfg):
    nc = bacc.Bacc("TRN2", target_bir_lowering=False, debug=False,
                   num_devices=cfg.n_cores)
    PD = cfg.pd
    n_all, npc, split, nsb, ncc = cfg.n_all, cfg.npc, cfg.split, cfg.nsb, cfg.ncc
    NBLK = GPB * (LOB + HIB)          # blocks per superblock (36)
    NLO = GPB * LO_CAP                # lo idxs per superblock (3072)
    NHI = GPB * HI_CAP                # hi idxs per superblock
    g_pad = nsb * GPB

    x_t = nc.dram_tensor("x", [n_all, IN_DIM], F32, kind="ExternalInput")
    w_t = nc.dram_tensor("w_pd", [IN_DIM, HEADS * OUT_DIM], PD, kind="ExternalInput")
    as4_t = nc.dram_tensor("as4_pd", [HEADS * OUT_DIM, 2 * HEADS], PD, kind="ExternalInput")
    idf_t = nc.dram_tensor("ident_f32", [128, 128], F32, kind="ExternalInput")
    idp_t = nc.dram_tensor("ident_pd", [128, 128], PD, kind="ExternalInput")
    iota_t = nc.dram_tensor("iota_pd", [128, WIN], PD, kind="ExternalInput")
    biasb_t = nc.dram_tensor("biasb", [128, HEADS * OUT_DIM], F32, kind="ExternalInput")
    il_t = nc.dram_tensor("idx_lo", [nsb, 128, NLO // 16], I16, kind="ExternalInput")
    ih_t = nc.dram_tensor("idx_hi", [nsb, 128, NHI // 16], I16, kind="ExternalInput")
    cx_t = nc.dram_tensor("colidx", [nsb, 128, NBLK], F16, kind="ExternalInput")
    adp_t = nc.dram_tensor("adst_pe", [nsb, 128, NBLK * HEADS], F16,
                           kind="ExternalInput")
    pi_t = nc.dram_tensor("permidx", [ncc, 128, PC_CHUNK // 16], I16, kind="ExternalInput")
    out_t = nc.dram_tensor("out", [npc, HEADS * OUT_DIM], F32, kind="ExternalOutput")

    table = nc.dram_tensor("table", [n_all, ROW_F16], PD, kind="Internal")
    scratch = nc.dram_tensor("scratch", [g_pad * WIN, 128], F32, kind="Internal")

    with tile.TileContext(nc) as tc:
        with tc.tile_pool(name="consts", bufs=1) as cpool:
            w_c = cpool.tile([IN_DIM, HEADS * OUT_DIM], PD)
            nc.sync.dma_start(w_c[:], w_t.ap())
            as4_c = cpool.tile([HEADS * OUT_DIM, 2 * HEADS], PD)
            nc.sync.dma_start(as4_c[:], as4_t.ap())
            idf_c = cpool.tile([128, 128], F32)
            nc.sync.dma_start(idf_c[:], idf_t.ap())
            idp_c = cpool.tile([128, 128], PD)
            nc.sync.dma_start(idp_c[:], idp_t.ap())
            iota_c = cpool.tile([128, WIN], PD)
            nc.sync.dma_start(iota_c[:], iota_t.ap())
            biasb_c = cpool.tile([128, HEADS * OUT_DIM], F32)
            nc.sync.dma_start(biasb_c[:], biasb_t.ap())

            table_f32 = table.ap().bitcast(F32)     # [n_all, 128]

            # ---------------- phase A: projection table ----------------
            n_tiles = (n_all + 127) // 128 if "A" in cfg.phases else 0
            with (tc.tile_pool(name="pa_sb", bufs=3) as pa,
                  tc.tile_pool(name="pa_ps", bufs=2, space="PSUM") as pps,
                  tc.tile_pool(name="pa_ps2", bufs=1, space="PSUM") as pps2):
                for t in range(n_tiles):
                    n0 = t * 128
                    p = min(128, n_all - n0)
                    xt = pa.tile([128, IN_DIM], F32, tag="xt")
                    nc.sync.dma_start(xt[:p, :], x_t.ap()[n0:n0 + p, :])
                    xT_ps = pps.tile([128, 128], F32, tag="xT")
                    nc.tensor.transpose(out=xT_ps[:, :p], in_=xt[:p, :],
                                        identity=idf_c[:p, :p])
                    xT = pa.tile([128, 128], PD, tag="xTs")
                    nc.scalar.copy(xT[:, :p], xT_ps[:, :p])
                    xpT_ps = pps.tile([128, 128], F32, tag="xpT")
                    nc.tensor.matmul(xpT_ps[:, :p], lhsT=w_c[:], rhs=xT[:, :p],
                                     start=True, stop=True)
                    xpT = pa.tile([128, 128], PD, tag="xpTs")
                    nc.scalar.copy(xpT[:, :p], xpT_ps[:, :p])
                    asd_ps = pps2.tile([2 * HEADS, 128], F32, tag="asd")
                    nc.tensor.matmul(asd_ps[:, :p], lhsT=as4_c[:], rhs=xpT[:, :p],
                                     start=True, stop=True)
                    rows_ps = pps.tile([128, 128], PD, tag="rows")
                    nc.tensor.transpose(out=rows_ps[:p, :], in_=xpT[:, :p],
                                        identity=idp_c[:, :])
                    rows = pa.tile([128, ROW_F16 // 2 + 1], PD, tag="rowsb")
                    nc.vector.tensor_copy(rows[:p, 0:128], rows_ps[:p, :])
                    nc.vector.memset(rows[:p, 128:129], 1.0)
                    asd = pa.tile([2 * HEADS, 128], F32, tag="asds")
                    nc.scalar.copy(asd[:, :p], asd_ps[:, :p])
                    asdT_ps = pps2.tile([128, 2 * HEADS], F32, tag="asdT")
                    nc.tensor.transpose(out=asdT_ps[:p, :], in_=asd[:, :p],
                                        identity=idf_c[:2 * HEADS, :2 * HEADS])
                    asdT = pa.tile([128, 2 * HEADS], F32, tag="asdTs")
                    nc.vector.tensor_copy(asdT[:p, :], asdT_ps[:p, :])
                    nc.sync.dma_start(table.ap()[n0:n0 + p, 0:129], rows[:p, :])
                    nc.sync.dma_start(table_f32[n0:n0 + p, 65:65 + 2 * HEADS],
                                      asdT[:p, :])

            # ---------------- phase B: gather + aggregate ----------------
            with (tc.tile_pool(name="pb_g", bufs=2) as gp,
                  tc.tile_pool(name="pb_m", bufs=2) as mp,
                  tc.tile_pool(name="pb_s", bufs=2) as sp,
                  tc.tile_pool(name="pb_z", bufs=3) as zp,
                  tc.tile_pool(name="pb_ps", bufs=8, space="PSUM") as up):
                BL = cfg.blevel
                for sb in range(nsb if "B" in cfg.phases else 0):
                    il = sp.tile([128, NLO // 16], I16, tag="il")
                    nc.sync.dma_start(il[:], il_t.ap()[sb])
                    ih = sp.tile([128, NHI // 16], I16, tag="ih")
                    nc.sync.dma_start(ih[:], ih_t.ap()[sb])
                    cx = sp.tile([128, NBLK], F16, tag="cx")
                    nc.sync.dma_start(cx[:], cx_t.ap()[sb])
                    adp = sp.tile([128, NBLK, HEADS], F16, tag="adp")
                    nc.sync.dma_start(adp[:].rearrange("p b h -> p (b h)"),
                                      adp_t.ap()[sb])

                    glo = gp.tile([128, GPB * LOB, ROW_F16], PD, tag="glo")
                    nc.gpsimd.dma_gather(glo[:], table.ap()[0:split, :], il[:],
                                         NLO, NLO, ROW_F16, single_packet=False)
                    ghi = gp.tile([128, GPB * HIB, ROW_F16], PD, tag="ghi")
                    nc.gpsimd.dma_gather(ghi[:], table.ap()[split:n_all, :], ih[:],
                                         NHI, NHI, ROW_F16, single_packet=False)

                    if BL < 2:
                        continue
                    # onehot[e, b, c] = (iota[c] == colidx[e, b])
                    oneh = mp.tile([128, NBLK, WIN], PD, tag="oneh")
                    nc.vector.tensor_tensor(
                        out=oneh[:],
                        in0=iota_c[:].unsqueeze(1).to_broadcast([128, NBLK, WIN]),
                        in1=cx[:].unsqueeze(2).to_broadcast([128, NBLK, WIN]),
                        op=mybir.AluOpType.is_equal)

                    # alpha = a_src[src] + a_dst[dst] ; lrelu ; exp
                    glo_f = glo[:].bitcast(F32)     # [128, GPB*LOB, 128]
                    ghi_f = ghi[:].bitcast(F32)
                    asum = sp.tile([128, NBLK, HEADS], F32, tag="asum")
                    nc.vector.tensor_tensor(out=asum[:, :GPB * LOB, :],
                                            in0=glo_f[:, :, 65:65 + HEADS],
                                            in1=adp[:, :GPB * LOB, :],
                                            op=mybir.AluOpType.add)
                    nc.vector.tensor_tensor(out=asum[:, GPB * LOB:, :],
                                            in0=ghi_f[:, :, 65:65 + HEADS],
                                            in1=adp[:, GPB * LOB:, :],
                                            op=mybir.AluOpType.add)
                    asc = sp.tile([128, NBLK, HEADS], F32, tag="asc")
                    nc.vector.tensor_scalar(asc[:], asum[:], LEAKY_SLOPE, None,
                                            mybir.AluOpType.mult)
                    alr = sp.tile([128, NBLK, HEADS], F32, tag="alr")
                    nc.vector.tensor_tensor(out=alr[:], in0=asum[:], in1=asc[:],
                                            op=mybir.AluOpType.max)
                    expa = sp.tile([128, NBLK, HEADS], PD, tag="expa")
                    nc.scalar.activation(expa[:], alr[:],
                                         mybir.ActivationFunctionType.Exp)

                    # Mw[e, b, h*32+c] = oneh[e, b, c] * expa[e, b, h]
                    mw = mp.tile([128, NBLK, HEADS, WIN], PD, tag="mw")
                    nc.vector.tensor_tensor(
                        out=mw[:],
                        in0=oneh[:].unsqueeze(2).to_broadcast([128, NBLK, HEADS, WIN]),
                        in1=expa[:].unsqueeze(3).to_broadcast([128, NBLK, HEADS, WIN]),
                        op=mybir.AluOpType.mult)

                    if BL < 3:
                        continue
                    for g in range(GPB):
                        U = up.tile([128, 130], F32, tag="U")
                        for j in range(LOB + HIB):
                            if j < LOB:
                                col = g * LOB + j
                                rhs = glo[:, col, 0:129]
                            else:
                                col = GPB * LOB + g * HIB + (j - LOB)
                                rhs = ghi[:, col - GPB * LOB, 0:129]
                            nc.tensor.matmul(U[:, 0:129], lhsT=mw[:, col, :, :],
                                             rhs=rhs, start=(j == 0),
                                             stop=(j == LOB + HIB - 1))
                        if BL < 4:
                            continue
                        rec = zp.tile([128, 1], F32, tag="rec")
                        nc.vector.reciprocal(rec[:], U[:, 128:129])
                        nz = zp.tile([WIN, 128], F32, tag="nz")
                        for h in range(HEADS):
                            sl = slice(h * WIN, (h + 1) * WIN)
                            nc.scalar.activation(nz[:, sl], U[sl, sl],
                                                 mybir.ActivationFunctionType.Copy,
                                                 scale=rec[sl, 0:1])
                        if cfg.bias_nonzero:
                            nc.vector.tensor_tensor(out=nz[:], in0=nz[:],
                                                    in1=biasb_c[:WIN, :],
                                                    op=mybir.AluOpType.add)
                        sg = zp.tile([WIN, 128], F32, tag="sg")
                        nc.scalar.activation(sg[:], nz[:],
                                             mybir.ActivationFunctionType.Sigmoid)
                        mix = zp.tile([WIN, 128], F32, tag="mix")
                        nc.vector.tensor_scalar(mix[:], sg[:], CMIX - BETA, BETA,
                                                mybir.AluOpType.mult,
                                                mybir.AluOpType.add)
                        orow = zp.tile([WIN, 128], F32, tag="orow")
                        nc.vector.tensor_tensor(out=orow[:], in0=nz[:], in1=mix[:],
                                                op=mybir.AluOpType.mult)
                        r0 = (sb * GPB + g) * WIN
                        nc.sync.dma_start(scratch.ap()[r0:r0 + WIN, :], orow[:])

            # ---------------- phase C: permute to node order ----------------
            with (tc.tile_pool(name="pc_s", bufs=2) as pcs,
                  tc.tile_pool(name="pc_g", bufs=2) as pcg):
                for k in range(ncc if "C" in cfg.phases else 0):
                    n0 = k * PC_CHUNK
                    valid = min(PC_CHUNK, npc - n0)
                    pidx = pcs.tile([128, PC_CHUNK // 16], I16, tag="pidx")
                    nc.sync.dma_start(pidx[:], pi_t.ap()[k])
                    gt = pcg.tile([128, PC_CHUNK // 128, 128], F32, tag="gt")
                    nc.gpsimd.dma_gather(gt[:], scratch.ap()[:, :], pidx[:],
                                         PC_CHUNK, valid, 128,
                                         single_packet=False)
                    nb = valid // 128
                    if nb:
                        nc.sync.dma_start(
                            out_t.ap()[n0:n0 + nb * 128, :]
                                 .rearrange("(b p) f -> p b f", p=128),
                            gt[:, 0:nb, :])
                    rem = valid - nb * 128
                    if rem:
                        nc.sync.dma_start(
                            out_t.ap()[n0 + nb * 128:n0 + valid, :],
                            gt[0:rem, nb, :])

    nc.compile()
    return nc


# ---------------------------------------------------------------- the API

def _make_const_inputs(W, att_src, att_dst, bias, pd_np):
    W = np.asarray(W, dtype=np.float32)
    att_src = np.asarray(att_src, dtype=np.float32)
    att_dst = np.asarray(att_dst, dtype=np.float32)
    bias = np.asarray(bias, dtype=np.float32)
    H, D = att_src.shape
    as4 = np.zeros((H * D, 2 * H), dtype=np.float32)
    for h in range(H):
        as4[h * D:(h + 1) * D, h] = att_src[h]
        as4[h * D:(h + 1) * D, H + h] = att_dst[h]
    ident = np.eye(128, dtype=np.float32)
    iota = np.tile(np.arange(WIN, dtype=np.float32), (128, 1))
    biasb = np.tile(bias, (128, 1)).astype(np.float32)
    return dict(w_pd=W.astype(pd_np), as4_pd=as4.astype(pd_np),
                ident_f32=ident, ident_pd=ident.astype(pd_np),
                iota_pd=iota.astype(pd_np), biasb=biasb)


def expand_adst(adstv, per_core, pd_np):
    """adstv [H, N] (device-computed) -> per-core adst_pe streams (host
    indexing only, no arithmetic)."""
    outs = []
    for c in per_core:
        dn = c["dstn"]                                  # [nsb, 128, NBLK]
        a = adstv[:, dn]                                # [H, nsb, 128, NBLK]
        a = np.moveaxis(a, 0, -1)                       # [nsb, 128, NBLK, H]
        nsb = a.shape[0]
        outs.append(np.ascontiguousarray(
            a.reshape(nsb, 128, -1).astype(pd_np)))
    return outs


def run(x, edge_index, W, att_src, att_dst, bias,
        n_all=N_NODES, n_cores=N_CORES, split=SPLIT, pd=F16, trace=False):
    npc = n_all // n_cores
    nsb, ncc, g_pad, per_core = preprocess(edge_index, n_all, npc, split, n_cores)
    bias_nonzero = bool(np.any(np.asarray(bias)))
    cfg = Cfg(n_all, npc, split, nsb, ncc, pd=pd, bias_nonzero=bias_nonzero,
              n_cores=n_cores)

    if pd == F16:
        pd_np = np.float16
    else:
        import ml_dtypes
        pd_np = ml_dtypes.bfloat16
    consts = _make_const_inputs(W, att_src, att_dst, bias, pd_np)
    x = np.ascontiguousarray(np.asarray(x, dtype=np.float32))
    W32 = np.asarray(W, dtype=np.float32)
    att_dst32 = np.asarray(att_dst, dtype=np.float32)
    H, D = att_dst32.shape
    ad4 = np.zeros((H * D, H), dtype=np.float32)
    for h in range(H):
        ad4[h * D:(h + 1) * D, h] = att_dst32[h]
    wad = (W32 @ ad4).astype(pd_np)                     # param-only host matmul

    # launch 1: per-core a_dst slab
    nc1 = build_nc_adst(npc, pd, n_cores)
    in_maps1 = [dict(x_slab=np.ascontiguousarray(x[c * npc:(c + 1) * npc]),
                     wad_pd=wad, ident_f32=consts["ident_f32"])
                for c in range(n_cores)]
    res1 = run_bass_kernel_spmd(nc1, in_maps1, core_ids=list(range(n_cores)),
                                trace=trace)
    adstv = np.concatenate([res1.results[c]["adstv"] for c in range(n_cores)],
                           axis=1)                      # [H, n_all]
    adst_pes = expand_adst(adstv, per_core, pd_np)

    # launch 2: the full layer
    nc = build_nc(cfg)
    in_maps = []
    for c in range(n_cores):
        m = dict(consts)
        m["x"] = x
        m["idx_lo"] = per_core[c]["idx_lo"]
        m["idx_hi"] = per_core[c]["idx_hi"]
        m["colidx"] = per_core[c]["colidx"]
        m["adst_pe"] = adst_pes[c]
        m["permidx"] = per_core[c]["permidx"]
        in_maps.append(m)
    res = run_bass_kernel_spmd(nc, in_maps, core_ids=list(range(n_cores)),
                               trace=trace)
    out = np.concatenate([res.results[c]["out"] for c in range(n_cores)], axis=0)
    parts = dict(nc1=nc1, in_maps1=in_maps1, nc2=nc, in_maps2=in_maps,
                 res1=res1, res2=res, n_cores=n_cores)
    return out, parts


def bench_pjrt(nc, in_maps, n_cores, iters=20):
    """Time repeated executions of a prebuilt Bass module via PJRT (axon).
    Returns (min_s, all_times). Inputs are pre-staged on device; no donation."""
    import time as _time

    import jax
    from jax.sharding import Mesh, NamedSharding, PartitionSpec
    from jax.experimental.shard_map import shard_map

    import concourse.mybir as mybir_
    from concourse import bass2jax as b2j

    b2j.install_neuronx_cc_hook()
    partition_name = (nc.partition_id_tensor.name
                      if nc.partition_id_tensor else None)
    in_names, out_names, out_avals, zero_outs = [], [], [], []
    for alloc in nc.m.functions[0].allocations:
        if not isinstance(alloc, mybir_.MemoryLocationSet):
            continue
        name = alloc.memorylocations[0].name
        if alloc.kind == "ExternalInput":
            if name != partition_name:
                in_names.append(name)
        elif alloc.kind == "ExternalOutput":
            dt = mybir_.dt.np(alloc.dtype)
            out_avals.append(jax.core.ShapedArray(tuple(alloc.tensor_shape), dt))
            out_names.append(name)
            zero_outs.append(np.zeros(tuple(alloc.tensor_shape), dt))

    # the bind's in_names must cover ALL operands (inputs + zero-out bufs
    # + partition id) — neuronx_cc_hook asserts len(in_names) == n_operands.
    bind_names = list(in_names) + list(out_names)
    if partition_name is not None:
        bind_names.append(partition_name)

    def _body(*args):
        operands = list(args)
        if partition_name is not None:
            operands.append(b2j.partition_id_tensor())
        outs = b2j._bass_exec_p.bind(
            *operands, out_avals=tuple(out_avals), in_names=tuple(bind_names),
            out_names=tuple(out_names), lowering_input_output_aliases=(),
            sim_require_finite=True, sim_require_nnan=True, nc=nc)
        return tuple(outs)

    n_params = len(in_names)
    devices = jax.devices()[:n_cores]
    mesh = Mesh(np.asarray(devices), ("core",))
    spec = PartitionSpec("core")
    fn = jax.jit(shard_map(_body, mesh=mesh,
                           in_specs=(spec,) * (n_params + len(zero_outs)),
                           out_specs=(spec,) * len(out_names),
                           check_rep=False), keep_unused=True)
    sh = NamedSharding(mesh, spec)
    args = [jax.device_put(
                np.concatenate([in_maps[c][nm] for c in range(n_cores)], 0), sh)
            for nm in in_names]
    args += [jax.device_put(
                np.zeros((n_cores * z.shape[0], *z.shape[1:]), z.dtype), sh)
             for z in zero_outs]
    r = fn(*args)
    jax.block_until_ready(r)
    times = []
    for _ in range(iters):
        t0 = _time.perf_counter()
        r = fn(*args)
        jax.block_until_ready(r)
        times.append(_time.perf_counter() - t0)
    return min(times), times


def kernel(**inputs) -> np.ndarray:
    out, _ = run(inputs["x"], inputs["edge_index"], inputs["W"],
                 inputs["att_src"], inputs["att_dst"], inputs["bias"])
    return out



# revision 30
# speedup vs baseline: 1.0766x; 1.0766x over previous
# MixGAT layer (GATConv + beta-mix swish) on 8 Trainium2 NeuronCores.
#
# Strategy (dst-node sharding, per spec sharding_hint):
#  - Nodes partitioned across 8 cores by dst id; each core owns N/8 dst rows.
#  - Params (W, att, bias) replicated; each core computes the full projected
#    table xp = x @ W on-device (replicated compute beats collectives here).
#  - Phase A (device): xp (fp16), a_src/a_dst (f32) written to a 512B/row HBM
#    table:  [128 x fp16 xp | 1.0 fp16 | 4 x f32 a_src | 4 x f32 a_dst | pad].
#  - Phase B (device): edges sorted by dst, packed into 32-node groups of
#    9 static 128-edge blocks (6 "lo" + 3 "hi": dma_gather indices are int16,
#    so the table is addressed as two halves).  dma_gather pulls each edge's
#    512B row (edge -> partition).  alpha = lrelu(a_src+a_dst) -> exp on ACT.
#    A sparse per-block weight matrix Mw[e, h*32+c] = exp(alpha[e,h]) (c = dst
#    slot) is built with is_equal/mult on DVE; ONE matmul per block computes
#    both the weighted feature aggregation and the softmax denominators (the
#    baked-in 1.0 column), accumulating over the group's blocks in PSUM.
#    Postproc divides by the denominator and applies the beta-mix swish.
#  - Phase C (device): small gather permutes group-slot rows to node order.
#
# kernel(**inputs) is self-contained: preprocessing is pure numpy, the device
# kernel is built with bass/Tile and run via run_bass_kernel_spmd on cores 0-7.

import math

import numpy as np

import concourse.bass as bass
import concourse.mybir as mybir
import concourse.tile as tile
from concourse import bacc
from concourse.bass_utils import run_bass_kernel_spmd

F32 = mybir.dt.float32
F16 = mybir.dt.float16
I16 = mybir.dt.int16

# problem constants
N_NODES = 50000
IN_DIM = 128
HEADS = 4
OUT_DIM = 32
LEAKY_SLOPE = 0.2
BETA = 0.5
CMIX = 1.2
N_CORES = 8

# static schedule constants
WIN = 32          # dst nodes per group (PSUM slots = HEADS*WIN = 128)
BLK = 128         # edges per block (gather slots -> partitions)
LOB = 6           # lo blocks per group
HIB = 3           # hi blocks per group
GPB = 4           # groups per superblock
SPLIT = 32768     # int16-addressable table split
LO_CAP = LOB * BLK
HI_CAP = HIB * BLK
DEAD = 100.0      # colidx value for dead slots (never equals iota 0..31)
ROW_F16 = 256     # fp16 elements per 512B table row
PC_CHUNK = 2048   # phase C gather chunk (nodes)


class Cfg:
    def __init__(self, n_all, npc, split, nsb, ncc, pd=F16, bias_nonzero=False,
                 n_cores=N_CORES, phases="ABC", blevel=4):
        self.phases = phases
        self.blevel = blevel
        self.n_all = n_all          # total nodes (table rows)
        self.npc = npc              # nodes per core
        self.split = split
        self.nsb = nsb              # superblocks per core
        self.ncc = ncc              # phase C chunks
        self.pd = pd
        self.bias_nonzero = bias_nonzero
        self.n_cores = n_cores


# ---------------------------------------------------------------- host side

def build_nc_adst(n_rows, pd, n_cores):
    """Launch-1 mini kernel: adstv[4, n_rows] = (W @ AD4).T @ xT_slab per core.
    x arrives pre-transposed ([feat, node]) so no on-device transpose."""
    nc = bacc.Bacc("TRN2", target_bir_lowering=False, debug=False,
                   num_devices=n_cores)
    TW = 512          # av_ps [4, TW] f32 must fit a 2KB PSUM bank
    xs_t = nc.dram_tensor("xT_slab", [IN_DIM, n_rows], pd, kind="ExternalInput")
    wad_t = nc.dram_tensor("wad_pd", [IN_DIM, HEADS], pd, kind="ExternalInput")
    out_t = nc.dram_tensor("adstv", [HEADS, n_rows], F32, kind="ExternalOutput")
    with tile.TileContext(nc) as tc:
        with (tc.tile_pool(name="c", bufs=1) as cp,
              tc.tile_pool(name="s", bufs=3) as sp,
              tc.tile_pool(name="p2", bufs=3, space="PSUM") as pp2):
            wad_c = cp.tile([IN_DIM, HEADS], pd)
            nc.sync.dma_start(wad_c[:], wad_t.ap())
            for n0 in range(0, n_rows, TW):
                p = min(TW, n_rows - n0)
                xt8 = sp.tile([128, TW], pd, tag="xt")
                nc.sync.dma_start(xt8[:, :p], xs_t.ap()[:, n0:n0 + p])
                av_ps = pp2.tile([HEADS, TW], F32, tag="av")
                nc.tensor.matmul(av_ps[:, :p], lhsT=wad_c[:], rhs=xt8[:, :p],
                                 start=True, stop=True)
                av8 = sp.tile([HEADS, TW], F32, tag="av8")
                nc.vector.tensor_copy(av8[:, :p], av_ps[:, :p])
                nc.sync.dma_start(out_t.ap()[:, n0:n0 + p], av8[:, :p])
    nc.compile()
    return nc


def _wrap16(v):
    """idx vector [S*16] -> dma_gather idx layout [128, S]."""
    s = v.reshape(-1, 16).T                      # [16, S]
    return np.tile(s, (8, 1)).astype(np.int16)   # [128, S]


def preprocess(edge_index, n_all, npc, split, n_cores):
    """Build per-core static schedules. Returns (cfg-ish dict, per-core arrays)."""
    src = np.asarray(edge_index[0], dtype=np.int64)
    dst = np.asarray(edge_index[1], dtype=np.int64)
    loop = np.arange(n_all, dtype=np.int64)
    src = np.concatenate([src, loop])
    dst = np.concatenate([dst, loop])
    order = np.argsort(dst, kind="stable")
    src = src[order]
    dst = dst[order]

    core_bounds = np.searchsorted(dst, np.arange(n_cores + 1) * npc)
    cores = []
    for c in range(n_cores):
        b0, b1 = core_bounds[c], core_bounds[c + 1]
        s = src[b0:b1]
        d = (dst[b0:b1] - c * npc).astype(np.int64)
        lo_mask = s < split
        deg_lo = np.bincount(d[lo_mask], minlength=npc)
        deg_hi = np.bincount(d[~lo_mask], minlength=npc)
        # node -> edge range (d is sorted)
        seg_end = np.cumsum(np.bincount(d, minlength=npc))
        seg_start = seg_end - (deg_lo + deg_hi)

        # greedy 32-node groups under the static caps
        groups = []  # (n0, cnt)
        n = 0
        while n < npc:
            cnt, lo, hi = 0, 0, 0
            while (n + cnt < npc and cnt < WIN
                   and lo + deg_lo[n + cnt] <= LO_CAP
                   and hi + deg_hi[n + cnt] <= HI_CAP):
                lo += deg_lo[n + cnt]
                hi += deg_hi[n + cnt]
                cnt += 1
            assert cnt > 0, "single node exceeds block caps"
            groups.append((n, cnt))
            n += cnt
        g_cnt = len(groups)

        # per-group static block data
        idx_lo = np.zeros((g_cnt, LO_CAP), dtype=np.int64)
        idx_hi = np.zeros((g_cnt, HI_CAP), dtype=np.int64)
        col_lo = np.full((g_cnt, LO_CAP), DEAD, dtype=np.float16)
        col_hi = np.full((g_cnt, HI_CAP), DEAD, dtype=np.float16)
        dn_lo = np.zeros((g_cnt, LO_CAP), dtype=np.int64)
        dn_hi = np.zeros((g_cnt, HI_CAP), dtype=np.int64)
        perm = np.empty(npc, dtype=np.int64)
        for g, (n0, cnt) in enumerate(groups):
            e0, e1 = seg_start[n0], seg_end[n0 + cnt - 1]
            gs = s[e0:e1]
            gd = d[e0:e1] - n0
            m = gs < split
            nl = int(m.sum())
            nh = int((~m).sum())
            idx_lo[g, :nl] = gs[m]
            col_lo[g, :nl] = gd[m].astype(np.float16)
            dn_lo[g, :nl] = c * npc + n0 + gd[m]
            idx_hi[g, :nh] = gs[~m] - split
            col_hi[g, :nh] = gd[~m].astype(np.float16)
            dn_hi[g, :nh] = c * npc + n0 + gd[~m]
            perm[n0:n0 + cnt] = g * WIN + np.arange(cnt)
        cores.append(dict(g_cnt=g_cnt, idx_lo=idx_lo, idx_hi=idx_hi,
                          col_lo=col_lo, col_hi=col_hi, dn_lo=dn_lo,
                          dn_hi=dn_hi, perm=perm))

    g_max = max(c["g_cnt"] for c in cores)
    g_pad = ((g_max + GPB - 1) // GPB) * GPB
    nsb = g_pad // GPB
    ncc = (npc + PC_CHUNK - 1) // PC_CHUNK

    per_core = []
    for c in cores:
        g_cnt = c["g_cnt"]
        il = np.zeros((g_pad, LO_CAP), dtype=np.int64)
        ih = np.zeros((g_pad, HI_CAP), dtype=np.int64)
        cl = np.full((g_pad, LO_CAP), DEAD, dtype=np.float16)
        ch = np.full((g_pad, HI_CAP), DEAD, dtype=np.float16)
        dl = np.zeros((g_pad, LO_CAP), dtype=np.int64)
        dh = np.zeros((g_pad, HI_CAP), dtype=np.int64)
        il[:g_cnt] = c["idx_lo"]
        ih[:g_cnt] = c["idx_hi"]
        cl[:g_cnt] = c["col_lo"]
        ch[:g_cnt] = c["col_hi"]
        dl[:g_cnt] = c["dn_lo"]
        dh[:g_cnt] = c["dn_hi"]

        # gather idx tensors: [NSB, 128, S]
        idx_lo_t = np.stack([_wrap16(il[sb * GPB:(sb + 1) * GPB].reshape(-1))
                             for sb in range(nsb)])
        idx_hi_t = np.stack([_wrap16(ih[sb * GPB:(sb + 1) * GPB].reshape(-1))
                             for sb in range(nsb)])

        # colidx tensor: [NSB, 128, GPB*(LOB+HIB)]  (block cols: 24 lo | 12 hi)
        cl_b = cl.reshape(g_pad, LOB, BLK)
        ch_b = ch.reshape(g_pad, HIB, BLK)
        dl_b = dl.reshape(g_pad, LOB, BLK)
        dh_b = dh.reshape(g_pad, HIB, BLK)
        colidx = np.empty((nsb, 128, GPB * (LOB + HIB)), dtype=np.float16)
        dstn = np.empty((nsb, 128, GPB * (LOB + HIB)), dtype=np.int64)
        for sb in range(nsb):
            sl = slice(sb * GPB, (sb + 1) * GPB)
            colidx[sb, :, :GPB * LOB] = cl_b[sl].reshape(-1, BLK).T
            colidx[sb, :, GPB * LOB:] = ch_b[sl].reshape(-1, BLK).T
            dstn[sb, :, :GPB * LOB] = dl_b[sl].reshape(-1, BLK).T
            dstn[sb, :, GPB * LOB:] = dh_b[sl].reshape(-1, BLK).T

        # phase C perm idx: [NCC, 128, PC_CHUNK//16]
        permidx = np.empty((ncc, 128, PC_CHUNK // 16), dtype=np.int16)
        for k in range(ncc):
            v = np.full(PC_CHUNK, -1, dtype=np.int64)
            n0 = k * PC_CHUNK
            n1 = min(npc, n0 + PC_CHUNK)
            v[:n1 - n0] = c["perm"][n0:n1]
            permidx[k] = _wrap16(v)

        per_core.append(dict(idx_lo=idx_lo_t, idx_hi=idx_hi_t, colidx=colidx,
                             dstn=dstn, permidx=permidx))
    return nsb, ncc, g_pad, per_core


# -------------------------------------------------------------- device side

def build_nc(cfg: Cfg):
    nc = bacc.Bacc("TRN2", target_bir_lowering=False, debug=False,
                   num_devices=cfg.n_cores)
    PD = cfg.pd
    n_all, npc, split, nsb, ncc = cfg.n_all, cfg.npc, cfg.split, cfg.nsb, cfg.ncc
    NBLK = GPB * (LOB + HIB)          # blocks per superblock (36)
    NLO = GPB * LO_CAP                # lo idxs per superblock (3072)
    NHI = GPB * HI_CAP                # hi idxs per superblock
    g_pad = nsb * GPB

    WF = IN_DIM + 1 + 2 * HEADS      # [W | zero col | W@att_src | W@att_dst]
    x_t = nc.dram_tensor("xT_pd", [IN_DIM, n_all], PD, kind="ExternalInput")
    wf_t = nc.dram_tensor("wfull_pd", [IN_DIM, WF], PD, kind="ExternalInput")
    iota_t = nc.dram_tensor("iota_pd", [128, WIN], PD, kind="ExternalInput")
    biasb_t = nc.dram_tensor("biasb", [128, HEADS * OUT_DIM], F32, kind="ExternalInput")
    SWT = NLO // 16 + NHI // 16 + NBLK + NBLK * HEADS
    st_t = nc.dram_tensor("streams", [nsb, 128, SWT], I16, kind="ExternalInput")
    pi_t = nc.dram_tensor("permidx", [ncc, 128, PC_CHUNK // 16], I16, kind="ExternalInput")
    out_t = nc.dram_tensor("out", [npc, HEADS * OUT_DIM], F32, kind="ExternalOutput")

    table = nc.dram_tensor("table", [n_all, ROW_F16], PD, kind="Internal")
    scratch = nc.dram_tensor("scratch", [g_pad * WIN, 128], PD, kind="Internal")

    with tile.TileContext(nc) as tc:
        with tc.tile_pool(name="consts", bufs=1) as cpool:
            wf_c = cpool.tile([IN_DIM, WF], PD)
            nc.sync.dma_start(wf_c[:], wf_t.ap())
            iota_c = cpool.tile([128, WIN], PD)
            nc.sync.dma_start(iota_c[:], iota_t.ap())
            biasb_c = cpool.tile([128, HEADS * OUT_DIM], F32)
            nc.sync.dma_start(biasb_c[:], biasb_t.ap())

            # ---------------- phase A: projection table ----------------
            # x arrives pre-transposed ([feat, node]) so each 128-node tile is
            # ONE matmul (lhsT = xT slice): out[n, :] = [xp | 0 | a_src a_dst]
            # (a_src/a_dst as f16 at cols 129:137).  8 tiles are batched per
            # x-load / table-write DMA: each dma_start holds the shared HWDGE
            # descriptor unit ~625ns, so DMA instruction count rules.
            n_tiles = (n_all + 127) // 128 if "A" in cfg.phases else 0
            T8 = 8
            n_big = n_all // (128 * T8) if "A" in cfg.phases else 0
            with (tc.tile_pool(name="pa_sb", bufs=3) as pa,
                  tc.tile_pool(name="pa_ps2", bufs=4, space="PSUM") as pps2):
                for it in range(n_big):
                    n0 = it * 128 * T8
                    xt8 = pa.tile([128, T8 * 128], PD, tag="xt")
                    nc.sync.dma_start(xt8[:], x_t.ap()[:, n0:n0 + 128 * T8])
                    rows8 = pa.tile([128, T8, ROW_F16], PD, tag="rowsb")
                    for j in range(T8):
                        ps = pps2.tile([128, WF], F32, tag="ps")
                        nc.tensor.matmul(ps[:],
                                         lhsT=xt8[:, j * 128:(j + 1) * 128],
                                         rhs=wf_c[:], start=True, stop=True)
                        nc.vector.tensor_copy(rows8[:, j, 0:WF], ps[:])
                    nc.vector.memset(rows8[:, :, 128:129], 1.0)
                    nc.sync.dma_start(
                        table.ap()[n0:n0 + 128 * T8, :]
                             .rearrange("(j p) f -> p j f", p=128),
                        rows8[:])
                for t in range(n_big * T8, n_tiles):
                    n0 = t * 128
                    p = min(128, n_all - n0)
                    xt = pa.tile([128, 128], PD, tag="xtt")
                    nc.sync.dma_start(xt[:, :p], x_t.ap()[:, n0:n0 + p])
                    ps = pps2.tile([128, WF], F32, tag="ps")
                    nc.tensor.matmul(ps[:p, :], lhsT=xt[:, :p], rhs=wf_c[:],
                                     start=True, stop=True)
                    rows = pa.tile([128, ROW_F16], PD, tag="rowst")
                    nc.vector.tensor_copy(rows[:p, 0:WF], ps[:p, :])
                    nc.vector.memset(rows[:p, 128:129], 1.0)
                    nc.sync.dma_start(table.ap()[n0:n0 + p, :], rows[:p, :])

            # ---------------- phase B: gather + aggregate ----------------
            with (tc.tile_pool(name="pb_g", bufs=3) as gp,
                  tc.tile_pool(name="pb_m", bufs=3) as mp,
                  tc.tile_pool(name="pb_s", bufs=4) as sp,
                  tc.tile_pool(name="pb_z", bufs=3) as zp,
                  tc.tile_pool(name="pb_ps", bufs=8, space="PSUM") as up):
                BL = cfg.blevel
                SW0 = NLO // 16
                SW1 = SW0 + NHI // 16
                SW2 = SW1 + NBLK
                SW = SW2 + NBLK * HEADS
                for sb in range(nsb if "B" in cfg.phases else 0):
                    strm = sp.tile([128, SW], I16, tag="strm")
                    nc.sync.dma_start(strm[:], st_t.ap()[sb])
                    il = strm[:, 0:SW0]
                    ih = strm[:, SW0:SW1]
                    cx = strm[:, SW1:SW2].bitcast(F16)
                    adp = (strm[:, SW2:SW].bitcast(F16)
                           .rearrange("p (b h) -> p b h", h=HEADS))

                    glo = gp.tile([128, GPB * LOB, ROW_F16], PD, tag="glo")
                    nc.gpsimd.dma_gather(glo[:], table.ap()[0:split, :], il,
                                         NLO, NLO, ROW_F16, single_packet=False)
                    ghi = gp.tile([128, GPB * HIB, ROW_F16], PD, tag="ghi")
                    nc.gpsimd.dma_gather(ghi[:], table.ap()[split:n_all, :], ih,
                                         NHI, NHI, ROW_F16, single_packet=False)

                    if BL < 2:
                        continue
                    # onehot[e, b, c] = (iota[c] == colidx[e, b])
                    oneh = mp.tile([128, NBLK, WIN], PD, tag="oneh")
                    nc.vector.tensor_tensor(
                        out=oneh[:],
                        in0=iota_c[:].unsqueeze(1).to_broadcast([128, NBLK, WIN]),
                        in1=cx.unsqueeze(2).to_broadcast([128, NBLK, WIN]),
                        op=mybir.AluOpType.is_equal)

                    # alpha = a_src[src] + a_dst[dst] ; lrelu ; exp
                    asum = sp.tile([128, NBLK, HEADS], F32, tag="asum")
                    nc.vector.tensor_tensor(out=asum[:, :GPB * LOB, :],
                                            in0=glo[:, :, 129:129 + HEADS],
                                            in1=adp[:, :GPB * LOB, :],
                                            op=mybir.AluOpType.add)
                    nc.vector.tensor_tensor(out=asum[:, GPB * LOB:, :],
                                            in0=ghi[:, :, 129:129 + HEADS],
                                            in1=adp[:, GPB * LOB:, :],
                                            op=mybir.AluOpType.add)
                    asc = sp.tile([128, NBLK, HEADS], F32, tag="asc")
                    nc.vector.tensor_scalar(asc[:], asum[:], LEAKY_SLOPE, None,
                                            mybir.AluOpType.mult)
                    alr = sp.tile([128, NBLK, HEADS], F32, tag="alr")
                    nc.vector.tensor_tensor(out=alr[:], in0=asum[:], in1=asc[:],
                                            op=mybir.AluOpType.max)
                    expa = sp.tile([128, NBLK, HEADS], PD, tag="expa")
                    nc.scalar.activation(expa[:], alr[:],
                                         mybir.ActivationFunctionType.Exp)

                    # Mw[e, b, h*32+c] = oneh[e, b, c] * expa[e, b, h]
                    mw = mp.tile([128, NBLK, HEADS, WIN], PD, tag="mw")
                    nc.vector.tensor_tensor(
                        out=mw[:],
                        in0=oneh[:].unsqueeze(2).to_broadcast([128, NBLK, HEADS, WIN]),
                        in1=expa[:].unsqueeze(3).to_broadcast([128, NBLK, HEADS, WIN]),
                        op=mybir.AluOpType.mult)

                    if BL < 3:
                        continue
                    zn_all = zp.tile([128, GPB, 128], PD, tag="znall")
                    for g in range(GPB):
                        U = up.tile([128, 130], F32, tag="U")
                        for j in range(LOB + HIB):
                            if j < LOB:
                                col = g * LOB + j
                                rhs = glo[:, col, 0:129]
                            else:
                                col = GPB * LOB + g * HIB + (j - LOB)
                                rhs = ghi[:, col - GPB * LOB, 0:129]
                            nc.tensor.matmul(U[:, 0:129], lhsT=mw[:, col, :, :],
                                             rhs=rhs, start=(j == 0),
                                             stop=(j == LOB + HIB - 1))
                        if BL < 4:
                            continue
                        # softmax-normalize only; swish moves to phase C
                        # (node space = 4x fewer elements, idle engines there)
                        rec = zp.tile([128, 1], F32, tag="rec")
                        nc.vector.reciprocal(rec[:], U[:, 128:129])
                        nc.scalar.activation(zn_all[:, g, :], U[:, 0:128],
                                             mybir.ActivationFunctionType.Copy,
                                             scale=rec[:, 0:1])
                    if BL < 4:
                        continue
                    # Per-head diagonal extraction on DVE, then ONE batched
                    # scratch write for the whole superblock (128 node rows).
                    nz2 = zp.tile([WIN, GPB, HEADS, WIN], PD, tag="nz2")
                    for h in range(HEADS):
                        nc.vector.tensor_copy(
                            nz2[:, :, h, :],
                            zn_all[h * WIN:(h + 1) * WIN, :,
                                   h * WIN:(h + 1) * WIN])
                    r0 = sb * GPB * WIN
                    nc.sync.dma_start(
                        scratch.ap()[r0:r0 + GPB * WIN, :]
                               .rearrange("(g c) (h k) -> c g h k",
                                          c=WIN, k=WIN),
                        nz2[:])

            # ------- phase C: permute to node order + beta-mix swish -------
            with (tc.tile_pool(name="pc_s", bufs=2) as pcs,
                  tc.tile_pool(name="pc_g", bufs=2) as pcg):
                for k in range(ncc if "C" in cfg.phases else 0):
                    n0 = k * PC_CHUNK
                    valid = min(PC_CHUNK, npc - n0)
                    pidx = pcs.tile([128, PC_CHUNK // 16], I16, tag="pidx")
                    nc.sync.dma_start(pidx[:], pi_t.ap()[k])
                    gt = pcg.tile([128, PC_CHUNK // 128, 128], PD, tag="gt")
                    nc.gpsimd.dma_gather(gt[:], scratch.ap()[:, :], pidx[:],
                                         PC_CHUNK, valid, 128,
                                         single_packet=False)
                    if cfg.bias_nonzero:
                        nc.vector.tensor_tensor(
                            out=gt[:], in0=gt[:],
                            in1=biasb_c[:].unsqueeze(1)
                                .to_broadcast([128, PC_CHUNK // 128, 128]),
                            op=mybir.AluOpType.add)
                    sg = pcg.tile([128, PC_CHUNK // 128, 128], PD, tag="sg")
                    nc.scalar.activation(sg[:], gt[:],
                                         mybir.ActivationFunctionType.Sigmoid)
                    mix = pcg.tile([128, PC_CHUNK // 128, 128], PD, tag="mix")
                    nc.vector.tensor_scalar(mix[:], sg[:], CMIX - BETA, BETA,
                                            mybir.AluOpType.mult,
                                            mybir.AluOpType.add)
                    orow = pcg.tile([128, PC_CHUNK // 128, 128], F32, tag="or")
                    nc.vector.tensor_tensor(out=orow[:], in0=gt[:], in1=mix[:],
                                            op=mybir.AluOpType.mult)
                    nb = valid // 128
                    if nb:
                        nc.sync.dma_start(
                            out_t.ap()[n0:n0 + nb * 128, :]
                                 .rearrange("(b p) f -> p b f", p=128),
                            orow[:, 0:nb, :])
                    rem = valid - nb * 128
                    if rem:
                        nc.sync.dma_start(
                            out_t.ap()[n0 + nb * 128:n0 + valid, :],
                            orow[0:rem, nb, :])

    nc.compile()
    return nc


# ---------------------------------------------------------------- the API

def _make_const_inputs(W, att_src, att_dst, bias, pd_np):
    W = np.asarray(W, dtype=np.float32)
    att_src = np.asarray(att_src, dtype=np.float32)
    att_dst = np.asarray(att_dst, dtype=np.float32)
    bias = np.asarray(bias, dtype=np.float32)
    H, D = att_src.shape
    as4 = np.zeros((H * D, 2 * H), dtype=np.float32)
    for h in range(H):
        as4[h * D:(h + 1) * D, h] = att_src[h]
        as4[h * D:(h + 1) * D, H + h] = att_dst[h]
    wfull = np.zeros((H * D, H * D + 1 + 2 * H), dtype=np.float32)
    wfull[:, 0:H * D] = W
    wfull[:, H * D + 1:] = W @ as4        # param-only host matmul
    iota = np.tile(np.arange(WIN, dtype=np.float32), (128, 1))
    biasb = np.tile(bias, (128, 1)).astype(np.float32)
    return dict(wfull_pd=wfull.astype(pd_np),
                iota_pd=iota.astype(pd_np), biasb=biasb)


def expand_adst(adstv, per_core, pd_np):
    """adstv [H, N] (device-computed) -> per-core adst_pe streams (host
    indexing only, no arithmetic)."""
    outs = []
    for c in per_core:
        dn = c["dstn"]                                  # [nsb, 128, NBLK]
        a = adstv[:, dn]                                # [H, nsb, 128, NBLK]
        a = np.moveaxis(a, 0, -1)                       # [nsb, 128, NBLK, H]
        nsb = a.shape[0]
        outs.append(np.ascontiguousarray(
            a.reshape(nsb, 128, -1).astype(pd_np)))
    return outs


def run(x, edge_index, W, att_src, att_dst, bias,
        n_all=N_NODES, n_cores=N_CORES, split=SPLIT, pd=F16, trace=False):
    npc = n_all // n_cores
    nsb, ncc, g_pad, per_core = preprocess(edge_index, n_all, npc, split, n_cores)
    bias_nonzero = bool(np.any(np.asarray(bias)))
    cfg = Cfg(n_all, npc, split, nsb, ncc, pd=pd, bias_nonzero=bias_nonzero,
              n_cores=n_cores)

    if pd == F16:
        pd_np = np.float16
    else:
        import ml_dtypes
        pd_np = ml_dtypes.bfloat16
    consts = _make_const_inputs(W, att_src, att_dst, bias, pd_np)
    xT16 = np.ascontiguousarray(
        np.asarray(x, dtype=np.float32).astype(pd_np).T)   # [feat, node]
    W32 = np.asarray(W, dtype=np.float32)
    att_dst32 = np.asarray(att_dst, dtype=np.float32)
    H, D = att_dst32.shape
    ad4 = np.zeros((H * D, H), dtype=np.float32)
    for h in range(H):
        ad4[h * D:(h + 1) * D, h] = att_dst32[h]
    wad = (W32 @ ad4).astype(pd_np)                     # param-only host matmul

    # launch 1: per-core a_dst slab
    nc1 = build_nc_adst(npc, pd, n_cores)
    in_maps1 = [dict(xT_slab=np.ascontiguousarray(xT16[:, c * npc:(c + 1) * npc]),
                     wad_pd=wad)
                for c in range(n_cores)]
    res1 = run_bass_kernel_spmd(nc1, in_maps1, core_ids=list(range(n_cores)),
                                trace=trace)
    adstv = np.concatenate([res1.results[c]["adstv"] for c in range(n_cores)],
                           axis=1)                      # [H, n_all]
    adst_pes = expand_adst(adstv, per_core, pd_np)

    # launch 2: the full layer
    nc = build_nc(cfg)
    in_maps = []
    for c in range(n_cores):
        m = dict(consts)
        m["xT_pd"] = xT16
        m["streams"] = np.ascontiguousarray(np.concatenate(
            [per_core[c]["idx_lo"], per_core[c]["idx_hi"],
             per_core[c]["colidx"].view(np.int16),
             adst_pes[c].view(np.int16)], axis=2))
        m["permidx"] = per_core[c]["permidx"]
        in_maps.append(m)
    res = run_bass_kernel_spmd(nc, in_maps, core_ids=list(range(n_cores)),
                               trace=trace)
    out = np.concatenate([res.results[c]["out"] for c in range(n_cores)], axis=0)
    parts = dict(nc1=nc1, in_maps1=in_maps1, nc2=nc, in_maps2=in_maps,
                 res1=res1, res2=res, n_cores=n_cores)
    return out, parts


def bench_pjrt(nc, in_maps, n_cores, iters=20):
    """Time repeated executions of a prebuilt Bass module via PJRT (axon).
    Returns (min_s, all_times). Inputs are pre-staged on device; no donation."""
    import time as _time

    import jax
    from jax.sharding import Mesh, NamedSharding, PartitionSpec
    from jax.experimental.shard_map import shard_map

    import concourse.mybir as mybir_
    from concourse import bass2jax as b2j

    b2j.install_neuronx_cc_hook()
    partition_name = (nc.partition_id_tensor.name
                      if nc.partition_id_tensor else None)
    in_names, out_names, out_avals, zero_outs = [], [], [], []
    for alloc in nc.m.functions[0].allocations:
        if not isinstance(alloc, mybir_.MemoryLocationSet):
            continue
        name = alloc.memorylocations[0].name
        if alloc.kind == "ExternalInput":
            if name != partition_name:
                in_names.append(name)
        elif alloc.kind == "ExternalOutput":
            dt = mybir_.dt.np(alloc.dtype)
            out_avals.append(jax.core.ShapedArray(tuple(alloc.tensor_shape), dt))
            out_names.append(name)
            zero_outs.append(np.zeros(tuple(alloc.tensor_shape), dt))

    # the bind's in_names must cover ALL operands (inputs + zero-out bufs
    # + partition id) — neuronx_cc_hook asserts len(in_names) == n_operands.
    bind_names = list(in_names) + list(out_names)
    if partition_name is not None:
        bind_names.append(partition_name)

    def _body(*args):
        operands = list(args)
        if partition_name is not None:
            operands.append(b2j.partition_id_tensor())
        outs = b2j._bass_exec_p.bind(
            *operands, out_avals=tuple(out_avals), in_names=tuple(bind_names),
            out_names=tuple(out_names), lowering_input_output_aliases=(),
            sim_require_finite=True, sim_require_nnan=True, nc=nc)
        return tuple(outs)

    n_params = len(in_names)
    devices = jax.devices()[:n_cores]
    mesh = Mesh(np.asarray(devices), ("core",))
    spec = PartitionSpec("core")
    fn = jax.jit(shard_map(_body, mesh=mesh,
                           in_specs=(spec,) * (n_params + len(zero_outs)),
                           out_specs=(spec,) * len(out_names),
                           check_rep=False), keep_unused=True)
    sh = NamedSharding(mesh, spec)
    args = [jax.device_put(
                np.concatenate([in_maps[c][nm] for c in range(n_cores)], 0), sh)
            for nm in in_names]
    args += [jax.device_put(
                np.zeros((n_cores * z.shape[0], *z.shape[1:]), z.dtype), sh)
             for z in zero_outs]
    r = fn(*args)
    jax.block_until_ready(r)
    times = []
    for _ in range(iters):
        t0 = _time.perf_counter()
        r = fn(*args)
        jax.block_until_ready(r)
        times.append(_time.perf_counter() - t0)
    return min(times), times


def kernel(**inputs) -> np.ndarray:
    out, _ = run(inputs["x"], inputs["edge_index"], inputs["W"],
                 inputs["att_src"], inputs["att_dst"], inputs["bias"])
    return out

